# revision 1
# baseline (speedup 1.0000x reference)
"""GTN meta-path kernel for TRN2, 8 NeuronCores — fp8 datapath, v14.

Math (reference):
    Ap = A transposed to [E, N, N]
    a  = sum_e softmax(w1_0)[c,e] * Ap[e]      (per channel c)
    b  = sum_e softmax(w2_0)[c,e] * Ap[e]
    H  = a @ b
    twice:  H = normalize(H) @ gtconv(Ap, w)   (normalize = zero diag, col-scale)
    out = symmetrized mean over channels.

Sharding: channel-parallel — core c computes channel c end to end, then one
banded AllReduce over the 8 cores and a local symmetrization.

Datapath: A and all four mixes live in fp8 (e4m3); the three chained GEMMs
run in DoubleRow fp8 perf mode (2 k-tiles per pass).  Normalized
intermediates are scaled by 2048 so their ~1/N magnitudes sit in fp8 range
(normalize is scale-invariant, so no correction is needed until the final
GEMM, whose writeback folds 1/2048).  h2t / AllReduce / phase-6 loads are
bf16.  Everything else (transposed-chain structure, PSUM-packed mix layout
with d2d unpack, banded collectives, pipelined symmetrize) as the bf16
version.
"""

import numpy as np

N = 2048
E = 8
C = 8
P = 128
NCORES = 8

_PROGRAM = None


def _softmax_rows(w: np.ndarray) -> np.ndarray:
    """w: [C, E, 1, 1] -> softmax over E, float64 precision, returns [C, E]."""
    x = w.reshape(C, E).astype(np.float64)
    x = x - x.max(axis=1, keepdims=True)
    ex = np.exp(x)
    return ex / ex.sum(axis=1, keepdims=True)


def _build_program():
    import concourse.bacc as bacc
    import concourse.mybir as mybir
    import concourse.tile as tile
    from concourse.masks import make_identity

    f32 = mybir.dt.float32
    bf16 = mybir.dt.bfloat16
    fp8 = mybir.dt.float8e4
    AX = mybir.AxisListType.X
    MUL = mybir.AluOpType.mult
    ADD = mybir.AluOpType.add
    NE = mybir.AluOpType.not_equal
    COPY = mybir.ActivationFunctionType.Copy
    DR = mybir.MatmulPerfMode.DoubleRow

    nc = bacc.Bacc("TRN2")
    A3_ext = nc.dram_tensor("At3", [P, P, N], fp8, kind="ExternalInput")
    w4_ext = nc.dram_tensor("wblk4", [P, 2, P], fp8, kind="ExternalInput")
    out_ext = nc.dram_tensor("out", [N, N], f32, kind="ExternalOutput")

    with tile.TileContext(nc) as tc:
        with (
            tc.tile_pool(name="dram", bufs=1, space="DRAM") as dpool,
            tc.tile_pool(name="const", bufs=1) as cpool,
            tc.tile_pool(name="bigmv", bufs=1) as bigpool_mv,
        ):
            # all four mixes, psum-packed: row = kb*64 + q*16 + k16
            # quartered so unpacking can start before the whole mix finishes
            packed = [
                dpool.tile([N, N], fp8, name=f"packed{qt}") for qt in range(4)
            ]
            anat = dpool.tile([N, N], fp8)          # a in natural [i, kappa]
            nat = [dpool.tile([N, N], fp8, name=f"nat{q}") for q in range(1, 4)]
            # per-channel H''^T and allreduced sum; uneven AR bands
            # (640/640/640/128) so the last band is tiny and lands right
            # after GEMM3, shrinking the exposed tail
            h2t_full = dpool.tile([N, N], fp8, name="h2t")
            BANDS = [(0, 640), (640, 1280), (1280, 1920), (1920, 2048)]
            s_sh = [
                dpool.tile(
                    [hi - lo, N], fp8, addr_space="Shared", name=f"ssh{bi}"
                )
                for bi, (lo, hi) in enumerate(BANDS)
            ]

            # --- constants ---
            w4_sb = cpool.tile([P, 2, P], fp8)
            nc.sync.dma_start(out=w4_sb[:], in_=w4_ext[:])
            identb = cpool.tile([P, P], bf16)
            make_identity(nc, identb[:])
            id64 = cpool.tile([P, P], fp8)
            nc.scalar.activation(id64[:], identb[:], COPY, scale=1.0 / 64.0)
            ident8 = cpool.tile([P, P], fp8)
            make_identity(nc, ident8[:])
            # diag masks: masks[:, v, y] = 0 where y == p + v*128 else 1
            masks = cpool.tile([P, 4, 512], f32)
            nc.gpsimd.memset(masks[:], 1.0)
            for v in range(4):
                nc.gpsimd.affine_select(
                    out=masks[:, v],
                    in_=masks[:, v],
                    compare_op=NE,
                    fill=0.0,
                    base=v * P,
                    pattern=[[-1, 512]],
                    channel_multiplier=1,
                )

            # =========== Phase 1: all four mixes in one PE pass ===========
            # mv[0] = a^T lives outside the mix pools so the transposes can
            # interleave with mix matmuls (PE is mostly idle during the mix)
            mv = [
                bigpool_mv.tile([P, 16, N], fp8, tag="mv0", name="mva"),
                bigpool_mv.tile([P, 16, N], fp8, tag="mv1", name="mvb"),
            ]
            anat_v = anat[:].rearrange("(ib p) k -> p ib k", p=P)

            def build_mv0_quarter(ib4, lpool, lpsum):
                for kc in range(16):
                    ld = lpool.tile([P, 4, P], fp8, tag="ld", bufs=6)
                    leng = nc.sync if kc % 2 == 0 else nc.scalar
                    leng.dma_start(
                        out=ld[:],
                        in_=anat_v[
                            :, ib4 * 4 : (ib4 + 1) * 4,
                            kc * P : (kc + 1) * P,
                        ],
                    )
                    # fp8 transpose needs output element step 2: write
                    # even columns of a double-width PSUM tile
                    tp = lpsum.tile(
                        [P, 1024], fp8, tag=f"tp{kc % 2}", name="tp", bufs=2
                    )
                    tpv = tp[:].rearrange("p (c two) -> p c two", two=2)[
                        :, :, 0
                    ]
                    for g in range(4):
                        nc.tensor.transpose(
                            tpv[:, g * P : (g + 1) * P],
                            ld[:, g, :],
                            ident8[:],
                        )
                    if kc % 2 == 0:
                        nc.vector.tensor_copy(
                            out=mv[0][:, kc, ib4 * 512 : (ib4 + 1) * 512],
                            in_=tpv[:],
                        )
                    else:
                        nc.scalar.copy(
                            mv[0][:, kc, ib4 * 512 : (ib4 + 1) * 512],
                            tpv[:],
                        )

            with (
                tc.tile_pool(name="mix", bufs=4) as mpool,
                tc.tile_pool(name="mixst", bufs=8) as spool,
                tc.tile_pool(name="mixps", bufs=4, space="PSUM") as mpsum,
            ):
                # quarter-rotated order: quarter 3 mixes first so its
                # mv0 transpose pass leaves the critical path; GEMM1 then
                # only waits on the b-plane's final unpack chunk
                for ld4 in list(range(24, 32)) + list(range(24)):
                    a3t = mpool.tile([P, 4, N], fp8, tag="a3t")
                    nc.sync.dma_start(
                        out=a3t[:],
                        in_=A3_ext[4 * ld4 : 4 * ld4 + 4].rearrange(
                            "b p j -> p b j"
                        ),
                    )
                    for half in range(2):
                        bp = ld4 * 2 + half
                        qt, bpl = bp // 16, bp % 16
                        st = spool.tile([P, N], fp8, tag="st")
                        for jc in range(4):
                            pm = mpsum.tile([P, 512], f32, tag="pm")
                            # DoubleRow: pair dim = the two h blocks; the
                            # block-diagonal weight routes block h to
                            # output rows [64h, 64h+64)
                            nc.tensor.matmul(
                                pm[:],
                                lhsT=w4_sb[:],
                                rhs=a3t[
                                    :,
                                    2 * half : 2 * half + 2,
                                    jc * 512 : (jc + 1) * 512,
                                ],
                                start=True,
                                stop=True,
                                perf_mode=DR,
                            )
                            ceng = nc.vector if jc % 2 == 0 else nc.scalar
                            if jc % 2 == 0:
                                nc.vector.tensor_copy(
                                    out=st[:, jc * 512 : (jc + 1) * 512],
                                    in_=pm[:],
                                )
                            else:
                                nc.scalar.copy(
                                    st[:, jc * 512 : (jc + 1) * 512], pm[:]
                                )
                        weng = nc.scalar if half == 0 else nc.sync
                        weng.dma_start(
                            out=packed[qt][bpl * P : (bpl + 1) * P, :],
                            in_=st[:],
                        )
                    if ld4 % 2 == 1:
                        # unpack (d2d) the 4 bp rows finished by this and
                        # the previous iteration; interleaving smooths HBM
                        # and leaves only a tiny chunk after the last write
                        qt = (2 * ld4 + 1) // 16
                        b0 = (2 * ld4 - 2) % 16
                        pk5 = packed[qt][:].rearrange(
                            "(bp h q k) j -> bp h q k j", h=2, q=4, k=16
                        )
                        for q in range(2):
                            dst_plane = anat if q == 0 else nat[q - 1]
                            d5 = dst_plane[:].rearrange(
                                "(qt bp h k) j -> qt bp h k j",
                                qt=4, h=2, k=16,
                            )
                            for h in range(2):
                                eng = nc.gpsimd if (q + h) % 2 == 0 else (
                                    nc.sync if q == 0 else nc.scalar
                                )
                                eng.dma_start(
                                    out=d5[qt, b0 : b0 + 4, h],
                                    in_=pk5[b0 : b0 + 4, h, q],
                                )
                        if b0 == 12:
                            pass
                    if ld4 % 2 == 1 and (2 * ld4 - 2) % 16 == 12:
                        # anat quarter complete: transpose it into mv0
                        # while the mix continues (once per quarter)
                        build_mv0_quarter((2 * ld4 + 1) // 16, mpool, mpsum)

            # =========== Phases 2-4: three chained GEMMs ===========
            with (
                tc.tile_pool(name="gw", bufs=3) as gpool,
                tc.tile_pool(name="nrm", bufs=4) as npool,
                tc.tile_pool(name="gps", bufs=2, space="PSUM") as gpsum,
            ):
                def gemm(qi, rhs_res, out_res, normalize):
                    """Transposed-chain GEMM: out = mix_q^T @ rhs (DoubleRow).

                    qi: q index in packed (1=b, 2=g1, 3=g2).
                    rhs_res: SBUF-resident moving operand [P, 16, N] fp8.
                    out_res: SBUF [P, 16, N] fp8 (normalize) or None (evict
                        bf16 to h2t with 1/2048 fold).
                    """
                    for ms in range(16):
                        bts = gpool.tile([P, 16, P], fp8, tag="bts", bufs=16)
                        nc.sync.dma_start(
                            out=bts[:],
                            in_=nat[qi - 1][:].rearrange(
                                "(kc p) j -> p kc j", p=P
                            )[:, :, ms * P : (ms + 1) * P],
                        )
                        ps = [
                            gpsum.tile(
                                [P, 512], f32, tag=f"ps{ic}", name=f"ps{ic}"
                            )
                            for ic in range(4)
                        ]
                        dc = (ms * P) // 512
                        v = ms % 4
                        # ic-major (diag chunk first): each chunk's
                        # normalize reduce starts while later chunks matmul,
                        # shortening the post-matmul serial chain
                        ic_order = [dc] + [i for i in range(4) if i != dc]
                        if normalize:
                            degp = npool.tile([P, 4], f32, tag="degp")
                        for ic in ic_order:
                            for tp8 in range(8):
                                nc.tensor.matmul(
                                    ps[ic][:],
                                    lhsT=bts[:, 2 * tp8 : 2 * tp8 + 2, :],
                                    rhs=rhs_res[
                                        :,
                                        2 * tp8 : 2 * tp8 + 2,
                                        ic * 512 : (ic + 1) * 512,
                                    ],
                                    start=(tp8 == 0),
                                    stop=(tp8 == 7),
                                    perf_mode=DR,
                                )
                            if normalize:
                                if ic == dc:
                                    # zero diagonal in place + masked row-sum
                                    nc.vector.scalar_tensor_tensor(
                                        out=ps[dc][:],
                                        in0=ps[dc][:],
                                        scalar=1.0,
                                        in1=masks[:, v],
                                        op0=MUL,
                                        op1=MUL,
                                        accum_out=degp[:, dc : dc + 1],
                                    )
                                else:
                                    nc.vector.tensor_reduce(
                                        degp[:, ic : ic + 1], ps[ic][:],
                                        AX, ADD,
                                    )
                        if normalize:
                            degs = npool.tile([P, 1], f32, tag="degs")
                            nc.vector.tensor_reduce(degs[:], degp[:], AX, ADD)
                            dinv = npool.tile([P, 1], f32, tag="dinv")
                            nc.vector.reciprocal(dinv[:], degs[:])
                            # fp8 range trick: feed 2048*Hn to the next GEMM
                            dinv2 = npool.tile([P, 1], f32, tag="dinv2")
                            nc.scalar.activation(
                                dinv2[:], dinv[:], COPY, scale=2048.0
                            )
                            for ic in range(4):
                                dst = out_res[:, ms, ic * 512 : (ic + 1) * 512]
                                if ic % 2 == 0:
                                    nc.scalar.activation(
                                        dst, ps[ic][:], COPY, scale=dinv2[:]
                                    )
                                else:
                                    nc.vector.tensor_scalar(
                                        out=dst,
                                        in0=ps[ic][:],
                                        scalar1=dinv2[:],
                                        scalar2=None,
                                        op0=MUL,
                                    )
                        else:
                            for ic in range(4):
                                st = gpool.tile([P, 512], fp8, tag="fstage", bufs=8)
                                if ic % 2 == 0:
                                    nc.scalar.activation(
                                        st[:], ps[ic][:], COPY,
                                        scale=1.0 / 512.0,
                                    )
                                else:
                                    nc.vector.tensor_scalar(
                                        out=st[:],
                                        in0=ps[ic][:],
                                        scalar1=1.0 / 512.0,
                                        scalar2=None,
                                        op0=MUL,
                                    )
                                nc.sync.dma_start(
                                    out=h2t_full[
                                        ms * P : (ms + 1) * P,
                                        ic * 512 : (ic + 1) * 512,
                                    ],
                                    in_=st[:],
                                )

                # GEMM1: Ht = b^T a^T ; normalize -> Hnt in mv[1]
                gemm(1, mv[0], mv[1], normalize=True)

                # unpack g1/g2 now - overlaps GEMM1/2 compute (HBM is idle)
                ucnt = 0
                for q in range(2, 4):
                    d5 = nat[q - 1][:].rearrange(
                        "(qt bp h k) j -> qt bp h k j", qt=4, h=2, k=16
                    )
                    for qt in range(4):
                        pk5l = packed[qt][:].rearrange(
                            "(bp h q k) j -> bp h q k j", h=2, q=4, k=16
                        )
                        for h in range(2):
                            ueng = nc.gpsimd if ucnt % 2 == 0 else nc.sync
                            ueng.dma_start(
                                out=d5[qt, :, h], in_=pk5l[:, h, q]
                            )
                            ucnt += 1
                # GEMM2: H't = g1^T Hnt ; normalize -> H'nt (reuse mv0 slot)
                mv0b = bigpool_mv.tile([P, 16, N], fp8, tag="mv0")
                gemm(2, mv[1], mv0b, normalize=True)
                # GEMM3: H''t = g2^T H'nt -> h2t (bf16), g2 pre-scaled by 1/16
                gemm(3, mv0b, None, normalize=False)

                # ===== Phase 5: banded AllReduce, pipelined with GEMM3 =====
                for bi, (lo, hi) in enumerate(BANDS):
                    nc.gpsimd.collective_compute(
                        "AllReduce",
                        ADD,
                        replica_groups=[list(range(NCORES))],
                        ins=[h2t_full[lo:hi, :].opt()],
                        outs=[s_sh[bi].opt()],
                    )

                # ===== Phase 6: symmetrize out = S + S^T, (ms, b) =====
                # readiness: srow for row-chunk ms needs the AR band holding
                # rows [128ms, 128ms+128); colb for col-band b needs bands
                # covering rows [512b, 512b+512). Order by worst need.
                def row_band(r):
                    for bi, (lo, hi) in enumerate(BANDS):
                        if r < hi:
                            return bi
                    return len(BANDS) - 1

                srow_need = [row_band(ms * P + P - 1) for ms in range(16)]
                colb_need = [row_band(b * 512 + 511) for b in range(4)]
                pairs = sorted(
                    ((ms, b) for ms in range(16) for b in range(4)),
                    key=lambda p: (
                        max(srow_need[p[0]], colb_need[p[1]]), p[1], p[0],
                    ),
                )
                s_colvs = [
                    s_sh[bi][:].rearrange("(nb p) m -> p nb m", p=P)
                    for bi in range(len(BANDS))
                ]

                def load_cols(colb, b, ms):
                    """colb[:, nb] <- S rows [512b+128nb ..+128], split at
                    AR-band crossings (each 128-chunk is band-aligned)."""
                    runs = []  # (nb0, band, chunk0, count)
                    for nb in range(4):
                        r0 = 512 * b + 128 * nb
                        bi = row_band(r0 + 127)
                        ck = (r0 - BANDS[bi][0]) // P
                        if runs and runs[-1][1] == bi and                                 runs[-1][2] + runs[-1][3] == ck:
                            runs[-1][3] += 1
                        else:
                            runs.append([nb, bi, ck, 1])
                    for li, (nb0, bi, ck, cnt) in enumerate(runs):
                        eng = nc.scalar if li == 0 else nc.sync
                        eng.dma_start(
                            out=colb[:, nb0 : nb0 + cnt, :],
                            in_=s_colvs[bi][
                                :, ck : ck + cnt, ms * P : (ms + 1) * P
                            ],
                        )

                for pi, (ms, b) in enumerate(pairs):
                    sb_ms = srow_need[ms]
                    srow = gpool.tile([P, 512], fp8, tag="srow", bufs=5)
                    nc.sync.dma_start(
                        out=srow[:],
                        in_=s_sh[sb_ms][
                            ms * P - BANDS[sb_ms][0] : (ms + 1) * P
                            - BANDS[sb_ms][0],
                            b * 512 : (b + 1) * 512,
                        ],
                    )
                    colb = gpool.tile([P, 4, P], fp8, tag="colb", bufs=5)
                    load_cols(colb, b, ms)
                    # colb^T/64 via regular matmul against the scaled
                    # identity (also converts fp8 -> f32 psum)
                    pst = gpsum.tile(
                        [P, 512], f32, tag=f"ps{pi % 2}", name="pst"
                    )
                    for g in range(4):
                        nc.tensor.matmul(
                            pst[:, g * P : (g + 1) * P],
                            lhsT=colb[:, g, :],
                            rhs=id64[:],
                            start=True,
                            stop=True,
                        )
                    ost = gpool.tile([P, 512], f32, tag="ost", bufs=5)
                    nc.vector.scalar_tensor_tensor(
                        out=ost[:],
                        in0=srow[:],
                        scalar=1.0 / 64.0,
                        in1=pst[:],
                        op0=MUL,
                        op1=ADD,
                    )
                    oeng = nc.sync if pi % 2 == 0 else nc.scalar
                    oeng.dma_start(
                        out=out_ext[
                            ms * P : (ms + 1) * P, b * 512 : (b + 1) * 512
                        ],
                        in_=ost[:],
                    )


    nc.compile()
    return nc


def _get_program():
    global _PROGRAM
    if _PROGRAM is None:
        _PROGRAM = _build_program()
    return _PROGRAM


def _make_wblk(sws) -> np.ndarray:
    """Block-diagonal mix weights [128, 16*len(sws)].

    wblk[(x*8+e), (q*16+x)] = sws[q][e]  for x in 0..15.
    Partitions = (16 x, 8 e) matching the host-permuted A layout; out
    partitions = (q, 16 x).
    """
    wblk = np.zeros((P, 16 * len(sws)), np.float32)
    for q, sw in enumerate(sws):
        for x in range(16):
            wblk[x * 8 : (x + 1) * 8, q * 16 + x] = sw.astype(np.float32)
    return wblk


def _prep_inputs(A, w1_0, w2_0, w_1, w_2):
    import ml_dtypes

    swa = _softmax_rows(np.asarray(w1_0))
    swb = _softmax_rows(np.asarray(w2_0))
    sg1 = _softmax_rows(np.asarray(w_1))
    # mean/symmetrize fold (1/16) lives in the GEMM3 writeback scale: fp8
    # weights would hit subnormals if folded here
    sg2 = _softmax_rows(np.asarray(w_2))

    af8 = np.asarray(A, dtype=np.float32)[0].astype(
        ml_dtypes.float8_e4m3fn
    )  # [k,j,e]
    # At3[b, (k16 e), j] = A[16b+k16, j, e]
    at3 = np.ascontiguousarray(af8.transpose(0, 2, 1).reshape(P, P, N))
    in_maps = []
    for c in range(NCORES):
        w4 = _make_wblk([swa[c], swb[c], sg1[c], sg2[c]])
        w4dr = np.zeros((P, 2, P), np.float32)
        w4dr[:, 0, :64] = w4
        w4dr[:, 1, 64:] = w4
        in_maps.append(
            {"At3": at3, "wblk4": w4dr.astype(ml_dtypes.float8_e4m3fn)}
        )
    return in_maps


def kernel(A, w1_0, w2_0, w_1, w_2):
    from concourse.bass_utils import run_bass_kernel_spmd

    in_maps = _prep_inputs(A, w1_0, w2_0, w_1, w_2)
    nc = _get_program()
    res = run_bass_kernel_spmd(nc, in_maps, list(range(NCORES)))
    return np.asarray(res.results[0]["out"], dtype=np.float32)

